# revision 2
# baseline (speedup 1.0000x reference)
"""AttnDecoderRNN single-step kernel on 8 TRN2 NeuronCores (Bass/Tile SPMD).

Tensor-parallel sharding:
  - attn_w / encoder_outputs sharded over ML (4096 -> 512/core)
  - comb_w / w_ih / w_hh sharded over output rows (one 128-chunk per gate per core)
  - out_w sharded over vocab (128000 -> 16000/core)  [dominant 512MB stream]
  - tiny vectors (e0/h0/biases) replicated
Collectives: AR(attn partial sums+denom) -> AG(x) -> AG(h_new) -> AR(softmax denom).
All big matmuls run as float32r (FP22 multiply, fp32 accumulate) at full PE rate.
"""

import sys

sys.path.insert(0, "/opt/trn_rl_repo")

import numpy as np

import concourse.bacc as bacc
import concourse.bass as bass
import concourse.tile as tile
from concourse import mybir
from concourse.bass_utils import run_bass_kernel_spmd

F32 = mybir.dt.float32
F32R = mybir.dt.float32r

V, H, ML = 128000, 1024, 4096
NC = 8                      # cores
MLS = ML // NC              # 512  attn rows per core
HC = H // NC                # 128  hidden chunk per core
VS = V // NC                # 16000 vocab rows per core
NVT = 32                    # vocab tiles per core
VT = VS // NVT              # 500  logits per vocab tile
VBLK = 8                    # vocab superblocks (DMA granularity)
VTB = NVT // VBLK           # 4 tiles per superblock
WPOOL_BUFS = 8
PSUM_O_BUFS = 6

_prog_cache = {}


def _r(ap):
    return ap if ap.dtype == F32R else ap.bitcast(F32R)


def build_program():
    if "nc" in _prog_cache:
        return _prog_cache["nc"]

    nc = bacc.Bacc("TRN2", target_bir_lowering=False, debug=False, num_devices=NC)

    # ---- per-core external inputs (all float32) ----
    d_cat1 = nc.dram_tensor("cat1_cols", [128, 16], F32, kind="ExternalInput")
    d_awT = nc.dram_tensor("attn_wT", [2 * H, MLS], F32, kind="ExternalInput")
    d_ab = nc.dram_tensor("attn_b", [1, MLS], F32, kind="ExternalInput")
    d_enc = nc.dram_tensor("enc", [MLS, H], F32, kind="ExternalInput")
    d_cwT = nc.dram_tensor("comb_wT", [2 * H, HC], F32, kind="ExternalInput")
    d_cb = nc.dram_tensor("comb_b", [1, HC], F32, kind="ExternalInput")
    d_wihT = nc.dram_tensor("w_ihT", [H, 3 * HC], F32, kind="ExternalInput")
    d_whhT = nc.dram_tensor("w_hhT", [H, 3 * HC], F32, kind="ExternalInput")
    d_bih = nc.dram_tensor("b_ih", [1, 3 * HC], F32, kind="ExternalInput")
    d_bhh = nc.dram_tensor("b_hh", [1, 3 * HC], F32, kind="ExternalInput")
    d_h0c = nc.dram_tensor("h0_chunk", [1, HC], F32, kind="ExternalInput")
    d_owT = nc.dram_tensor("out_wT", [H, VS], F32, kind="ExternalInput")
    d_ob = nc.dram_tensor("out_b", [VS], F32, kind="ExternalInput")

    # ---- per-core external outputs ----
    o_attn = nc.dram_tensor("out_attn", [MLS], F32, kind="ExternalOutput")
    o_h = nc.dram_tensor("out_h", [HC], F32, kind="ExternalOutput")
    o_logp = nc.dram_tensor("out_logp", [VS], F32, kind="ExternalOutput")

    RG = [list(range(NC))]

    with tile.TileContext(nc) as tc:
        with (
            tc.tile_pool(name="stage", bufs=1) as sp,
            tc.tile_pool(name="small", bufs=1) as sm,
            tc.tile_pool(name="dram", bufs=1, space="DRAM") as dp,
            tc.tile_pool(name="lstage", bufs=4) as lp,
            tc.tile_pool(name="wpool", bufs=WPOOL_BUFS) as wp,
        ):
            # ============ stage inputs (small weights) -> SBUF ============
            cat1_sb = sp.tile([128, 16], F32R)
            nc.scalar.dma_start(cat1_sb[:], d_cat1[:].bitcast(F32R))
            ab_sb = sp.tile([1, MLS], F32)
            nc.scalar.dma_start(ab_sb[:], d_ab[:])
            cb_sb = sp.tile([1, HC], F32)
            nc.scalar.dma_start(cb_sb[:], d_cb[:])
            bih_sb = sp.tile([1, 3 * HC], F32)
            nc.scalar.dma_start(bih_sb[:], d_bih[:])
            bhh_sb = sp.tile([1, 3 * HC], F32)
            nc.scalar.dma_start(bhh_sb[:], d_bhh[:])
            h0c_sb = sp.tile([1, HC], F32)
            nc.scalar.dma_start(h0c_sb[:], d_h0c[:])

            aw_sb = sp.tile([128, 16, MLS], F32R)
            nc.sync.dma_start(aw_sb[:], d_awT[:].rearrange("(j p) n -> p j n", p=128).bitcast(F32R))
            enc_sb = sp.tile([128, 4, H], F32R)
            nc.sync.dma_start(enc_sb[:], d_enc[:].rearrange("(mc p) h -> p mc h", p=128).bitcast(F32R))
            cw_sb = sp.tile([128, 16, HC], F32R)
            nc.sync.dma_start(cw_sb[:], d_cwT[:].rearrange("(j p) n -> p j n", p=128).bitcast(F32R))
            wih_sb = sp.tile([128, 8, 3 * HC], F32R)
            nc.sync.dma_start(wih_sb[:], d_wihT[:].rearrange("(j p) n -> p j n", p=128).bitcast(F32R))
            whh_sb = sp.tile([128, 8, 3 * HC], F32R)
            nc.sync.dma_start(whh_sb[:], d_whhT[:].rearrange("(j p) n -> p j n", p=128).bitcast(F32R))

            # DRAM bounce buffers for collectives / repartitioning
            unn_dram = dp.tile([MLS], F32)
            ar1_in = dp.tile([1056], F32)
            ar1_out = dp.tile([1056], F32, addr_space="Shared")
            ag2_in = dp.tile([HC], F32)
            ag2_out = dp.tile([H], F32, addr_space="Shared")
            ag3_in = dp.tile([HC], F32)
            ag3_out = dp.tile([H], F32, addr_space="Shared")
            ar4_in = dp.tile([8], F32)
            ar4_out = dp.tile([8], F32, addr_space="Shared")
            scratch = dp.tile([VS], F32)

            with tc.tile_pool(name="psum14", bufs=1, space="PSUM") as pp:
                # ============ stage 1: attention scores ============
                # attn_logits[m] = sum_j cat1[j*128+p] * attn_wT[j*128+p, m]
                psum1 = pp.tile([1, MLS], F32)
                for j in range(16):
                    nc.tensor.matmul(
                        psum1[:], _r(cat1_sb[:, j : j + 1]), _r(aw_sb[:, j, :]),
                        start=(j == 0), stop=(j == 15),
                    )
                logits1 = sm.tile([1, MLS], F32)
                nc.vector.tensor_add(logits1[:], psum1[:], ab_sb[:])
                # softmax without max-subtraction (logits are O(1) by construction)
                unnorm = sm.tile([1, MLS], F32)
                d_part = sm.tile([1, 1], F32)
                nc.scalar.activation(
                    unnorm[:], logits1[:], mybir.ActivationFunctionType.Exp,
                    accum_out=d_part[:],
                )
                # repartition [1,512] -> [128,4] via DRAM bounce
                nc.scalar.dma_start(unn_dram[:].unsqueeze(0), unnorm[:])
                unn_cols = sm.tile([128, 4], F32R)
                nc.scalar.dma_start(
                    unn_cols[:], unn_dram[:].rearrange("(mc p) -> p mc", p=128).bitcast(F32R)
                )

                # ============ stage 2: attn_applied partials ============
                psum2a = pp.tile([1, 512], F32)
                psum2b = pp.tile([1, 512], F32)
                for mc in range(4):
                    nc.tensor.matmul(
                        psum2a[:], _r(unn_cols[:, mc : mc + 1]), _r(enc_sb[:, mc, 0:512]),
                        start=(mc == 0), stop=(mc == 3),
                    )
                for mc in range(4):
                    nc.tensor.matmul(
                        psum2b[:], _r(unn_cols[:, mc : mc + 1]), _r(enc_sb[:, mc, 512:1024]),
                        start=(mc == 0), stop=(mc == 3),
                    )
                # assemble AR1 input [u(1024) | d(1) | pad]
                ar1_sb = sm.tile([1, 1056], F32)
                nc.vector.memset(ar1_sb[:], 0.0)
                nc.vector.tensor_copy(ar1_sb[:, 0:512], psum2a[:])
                nc.vector.tensor_copy(ar1_sb[:, 512:1024], psum2b[:])
                nc.vector.tensor_copy(ar1_sb[:, 1024:1025], d_part[:])
                nc.scalar.dma_start(ar1_in[:].unsqueeze(0), ar1_sb[:])
                nc.gpsimd.collective_compute(
                    "AllReduce", mybir.AluOpType.add, replica_groups=RG,
                    ins=[ar1_in[:]], outs=[ar1_out[:]],
                )
                u_cols = sm.tile([128, 8], F32R)
                nc.scalar.dma_start(
                    u_cols[:], ar1_out[0:1024].rearrange("(j p) -> p j", p=128).bitcast(F32R)
                )
                dtot = sm.tile([1, 1], F32)
                nc.scalar.dma_start(dtot[:], ar1_out[1024:1025].unsqueeze(0))
                inv_d = sm.tile([1, 1], F32)
                nc.vector.reciprocal(inv_d[:], dtot[:])
                # attn_weights output = unnorm / d_total
                aw_out = sm.tile([1, MLS], F32)
                nc.vector.tensor_scalar_mul(aw_out[:], unnorm[:], inv_d[:])
                nc.scalar.dma_start(o_attn[:].unsqueeze(0), aw_out[:])

                # ============ stage 3: combine + relu ============
                # x_pre = [e0, u/d] @ comb_wT = A + (1/d) * B
                psumA = pp.tile([1, HC], F32)
                psumB = pp.tile([1, HC], F32)
                for j in range(8):
                    nc.tensor.matmul(
                        psumA[:], _r(cat1_sb[:, j : j + 1]), _r(cw_sb[:, j, :]),
                        start=(j == 0), stop=(j == 7),
                    )
                for j in range(8):
                    nc.tensor.matmul(
                        psumB[:], _r(u_cols[:, j : j + 1]), _r(cw_sb[:, j + 8, :]),
                        start=(j == 0), stop=(j == 7),
                    )
                xt1 = sm.tile([1, HC], F32)
                nc.vector.tensor_scalar_mul(xt1[:], psumB[:], inv_d[:])
                nc.vector.tensor_add(xt1[:], xt1[:], psumA[:])
                nc.vector.tensor_add(xt1[:], xt1[:], cb_sb[:])
                x_sb = sm.tile([1, HC], F32)
                nc.scalar.activation(x_sb[:], xt1[:], mybir.ActivationFunctionType.Relu)
                nc.scalar.dma_start(ag2_in[:].unsqueeze(0), x_sb[:])
                nc.gpsimd.collective_compute(
                    "AllGather", mybir.AluOpType.bypass, replica_groups=RG,
                    ins=[ag2_in[:]], outs=[ag2_out[:]],
                )
                x_cols = sm.tile([128, 8], F32R)
                nc.scalar.dma_start(
                    x_cols[:], ag2_out[:].rearrange("(j p) -> p j", p=128).bitcast(F32R)
                )

                # ============ stage 4: GRU step (one 128-chunk per gate) ============
                psum_gi = pp.tile([1, 3 * HC], F32)
                psum_gh = pp.tile([1, 3 * HC], F32)
                for j in range(8):
                    nc.tensor.matmul(
                        psum_gh[:], _r(cat1_sb[:, 8 + j : 9 + j]), _r(whh_sb[:, j, :]),
                        start=(j == 0), stop=(j == 7),
                    )
                for j in range(8):
                    nc.tensor.matmul(
                        psum_gi[:], _r(x_cols[:, j : j + 1]), _r(wih_sb[:, j, :]),
                        start=(j == 0), stop=(j == 7),
                    )
                gi = sm.tile([1, 3 * HC], F32)
                gh = sm.tile([1, 3 * HC], F32)
                nc.vector.tensor_add(gi[:], psum_gi[:], bih_sb[:])
                nc.vector.tensor_add(gh[:], psum_gh[:], bhh_sb[:])
                r_sb = sm.tile([1, HC], F32)
                z_sb = sm.tile([1, HC], F32)
                n_sb = sm.tile([1, HC], F32)
                t_sb = sm.tile([1, HC], F32)
                nc.vector.tensor_add(t_sb[:], gi[:, 0:HC], gh[:, 0:HC])
                nc.scalar.activation(r_sb[:], t_sb[:], mybir.ActivationFunctionType.Sigmoid)
                nc.vector.tensor_add(t_sb[:], gi[:, HC : 2 * HC], gh[:, HC : 2 * HC])
                nc.scalar.activation(z_sb[:], t_sb[:], mybir.ActivationFunctionType.Sigmoid)
                nc.vector.tensor_mul(t_sb[:], r_sb[:], gh[:, 2 * HC : 3 * HC])
                nc.vector.tensor_add(t_sb[:], t_sb[:], gi[:, 2 * HC : 3 * HC])
                nc.scalar.activation(n_sb[:], t_sb[:], mybir.ActivationFunctionType.Tanh)
                # h_new = n + z*(h0 - n)
                hnew = sm.tile([1, HC], F32)
                nc.vector.tensor_sub(hnew[:], h0c_sb[:], n_sb[:])
                nc.vector.tensor_mul(hnew[:], z_sb[:], hnew[:])
                nc.vector.tensor_add(hnew[:], n_sb[:], hnew[:])
                nc.scalar.dma_start(o_h[:].unsqueeze(0), hnew[:])
                nc.scalar.dma_start(ag3_in[:].unsqueeze(0), hnew[:])
                nc.gpsimd.collective_compute(
                    "AllGather", mybir.AluOpType.bypass, replica_groups=RG,
                    ins=[ag3_in[:]], outs=[ag3_out[:]],
                )
                h_cols = sm.tile([128, 8], F32R)
                nc.scalar.dma_start(
                    h_cols[:], ag3_out[:].rearrange("(j p) -> p j", p=128).bitcast(F32R)
                )

            # ============ stage 5: out-projection (the big stream) ============
            with tc.tile_pool(name="psum5", bufs=PSUM_O_BUFS, space="PSUM") as pp5:
                for vb in range(VBLK):
                    wts = []
                    for k in range(8):
                        w_t = wp.tile([128, VTB * VT], F32R, tag="w")
                        nc.sync.dma_start(
                            w_t[:],
                            d_owT[k * 128 : (k + 1) * 128,
                                  vb * VTB * VT : (vb + 1) * VTB * VT].bitcast(F32R),
                        )
                        wts.append(w_t)
                    for i in range(VTB):
                        v = vb * VTB + i
                        psum_o = pp5.tile([1, VT], F32, tag="po")
                        for k in range(8):
                            nc.tensor.matmul(
                                psum_o[:],
                                _r(h_cols[:, k : k + 1]),
                                _r(wts[k][:, i * VT : (i + 1) * VT]),
                                start=(k == 0), stop=(k == 7),
                            )
                        logit_sb = lp.tile([1, VT], F32, tag="ls")
                        nc.vector.tensor_copy(logit_sb[:], psum_o[:])
                        nc.scalar.dma_start(
                            scratch[v * VT : (v + 1) * VT].unsqueeze(0), logit_sb[:]
                        )

            # ============ tail: log-softmax over full vocab ============
            with tc.tile_pool(name="psumT", bufs=1, space="PSUM") as ppt:
                L128 = sm.tile([128, VS // 128], F32)
                nc.scalar.dma_start(L128[:], scratch[:].rearrange("(p j) -> p j", p=128))
                B128 = sm.tile([128, VS // 128], F32)
                nc.scalar.dma_start(B128[:], d_ob[:].rearrange("(p j) -> p j", p=128))
                nc.vector.tensor_add(L128[:], L128[:], B128[:])
                E128 = sm.tile([128, VS // 128], F32)
                esum = sm.tile([128, 1], F32)
                nc.scalar.activation(
                    E128[:], L128[:], mybir.ActivationFunctionType.Exp,
                    accum_out=esum[:],
                )
                ones_col = sm.tile([128, 1], F32)
                nc.vector.memset(ones_col[:], 1.0)
                psum_s = ppt.tile([1, 1], F32)
                nc.tensor.matmul(psum_s[:], ones_col[:], esum[:], start=True, stop=True)
                ar4_sb = sm.tile([1, 8], F32)
                nc.vector.memset(ar4_sb[:], 0.0)
                nc.vector.tensor_copy(ar4_sb[:, 0:1], psum_s[:])
                nc.scalar.dma_start(ar4_in[:].unsqueeze(0), ar4_sb[:])
                nc.gpsimd.collective_compute(
                    "AllReduce", mybir.AluOpType.add, replica_groups=RG,
                    ins=[ar4_in[:]], outs=[ar4_out[:]],
                )
                stot = sm.tile([1, 1], F32)
                nc.scalar.dma_start(stot[:], ar4_out[0:1].unsqueeze(0))
                lse = sm.tile([1, 1], F32)
                nc.scalar.activation(lse[:], stot[:], mybir.ActivationFunctionType.Ln)
                # broadcast lse to all 128 partitions via PE
                ones_row = sm.tile([1, 128], F32)
                nc.vector.memset(ones_row[:], 1.0)
                psum_b = ppt.tile([128, 1], F32)
                nc.tensor.matmul(psum_b[:], ones_row[:], lse[:], start=True, stop=True)
                lse128 = sm.tile([128, 1], F32)
                nc.vector.tensor_copy(lse128[:], psum_b[:])
                OUT128 = sm.tile([128, VS // 128], F32)
                nc.vector.tensor_scalar(
                    OUT128[:], L128[:], lse128[:], None, mybir.AluOpType.subtract
                )
                nc.scalar.dma_start(
                    o_logp[:].rearrange("(p j) -> p j", p=128), OUT128[:]
                )

    nc.compile()
    _prog_cache["nc"] = nc
    return nc


def make_in_maps(
    input_ids, hidden, encoder_outputs, emb, attn_w, attn_b,
    comb_w, comb_b, w_ih, w_hh, b_ih, b_hh, out_w, out_b,
):
    f = np.float32
    idx = int(np.asarray(input_ids).reshape(-1)[0])
    e0 = np.asarray(emb[idx], f).reshape(H)
    h0 = np.asarray(hidden, f).reshape(H)
    cat1 = np.concatenate([e0, h0])
    cat1_cols = np.ascontiguousarray(cat1.reshape(16, 128).T)  # [128,16]

    attn_w = np.asarray(attn_w, f)
    attn_b = np.asarray(attn_b, f)
    enc = np.asarray(encoder_outputs, f)
    comb_w = np.asarray(comb_w, f)
    comb_b = np.asarray(comb_b, f)
    w_ih = np.asarray(w_ih, f)
    w_hh = np.asarray(w_hh, f)
    b_ih = np.asarray(b_ih, f)
    b_hh = np.asarray(b_hh, f)
    out_w = np.asarray(out_w, f)
    out_b = np.asarray(out_b, f)

    in_maps = []
    for c in range(NC):
        ms = slice(c * MLS, (c + 1) * MLS)
        rows = np.r_[c * HC : (c + 1) * HC,
                     H + c * HC : H + (c + 1) * HC,
                     2 * H + c * HC : 2 * H + (c + 1) * HC]
        in_maps.append({
            "cat1_cols": cat1_cols,
            "attn_wT": np.ascontiguousarray(attn_w[ms].T),
            "attn_b": attn_b[ms].reshape(1, MLS),
            "enc": np.ascontiguousarray(enc[ms]),
            "comb_wT": np.ascontiguousarray(comb_w[c * HC : (c + 1) * HC].T),
            "comb_b": comb_b[c * HC : (c + 1) * HC].reshape(1, HC),
            "w_ihT": np.ascontiguousarray(w_ih[rows].T),
            "w_hhT": np.ascontiguousarray(w_hh[rows].T),
            "b_ih": b_ih[rows].reshape(1, 3 * HC),
            "b_hh": b_hh[rows].reshape(1, 3 * HC),
            "h0_chunk": h0[c * HC : (c + 1) * HC].reshape(1, HC),
            "out_wT": np.ascontiguousarray(out_w[c * VS : (c + 1) * VS].T),
            "out_b": np.ascontiguousarray(out_b[c * VS : (c + 1) * VS]),
        })
    return in_maps


def kernel(**inputs):
    nc = build_program()
    in_maps = make_in_maps(**inputs)
    res = run_bass_kernel_spmd(nc, in_maps, list(range(NC)))
    out = np.concatenate([res.results[c]["out_logp"] for c in range(NC)]).reshape(1, V)
    h_new = np.concatenate([res.results[c]["out_h"] for c in range(NC)]).reshape(1, 1, H)
    attn = np.concatenate([res.results[c]["out_attn"] for c in range(NC)]).reshape(1, ML)
    return out, h_new, attn


# revision 21
# speedup vs baseline: 14.1980x; 14.1980x over previous
"""AttnDecoderRNN single-step kernel on 8 TRN2 NeuronCores (Bass/Tile SPMD).

Tensor-parallel sharding:
  - attn_w / encoder_outputs sharded over ML (4096 -> 512/core)
  - comb_w sharded over output rows (x-chunk per core); GRU w_ih/w_hh sharded
    over the *input* dim so each core only needs its local x/h0 chunk
  - out_w sharded over vocab (128000 -> 16000/core)  [dominant 512MB stream]
Collectives (3): AR1(attn weighted-sum partials + denom, 4KB)
                 AR2(gi|gh partial sums, 24KB) -> full gates on every core
                 AR4(softmax denominator, 32B)
All heavy matmuls run as float32r (FP22 multiply, fp32 accumulate) at full
PE rate; host pre-tiles every weight shard so DMAs are fully contiguous.
"""

import sys

sys.path.insert(0, "/opt/trn_rl_repo")

import numpy as np

import concourse.bacc as bacc
import concourse.bass as bass
import concourse.tile as tile
from concourse import mybir
from concourse.tile import add_dep_helper
from concourse.bass_utils import run_bass_kernel_spmd

F32 = mybir.dt.float32
F32R = mybir.dt.float32r
AF = mybir.ActivationFunctionType

V, H, ML = 128000, 1024, 4096
NC = 8                      # cores
MLS = ML // NC              # 512  attn rows per core
HC = H // NC                # 128  hidden chunk per core
VS = V // NC                # 16000 vocab rows per core
NVT = 32                    # vocab tiles per core
VT = VS // NVT              # 500  logits per vocab tile
VBLK = 8                    # vocab superblocks (DMA granularity)
VTB = NVT // VBLK           # 4 tiles per superblock
WPOOL_BUFS = 15
PSUM_O_BUFS = 8

_prog_cache = {}


def build_program(spmd=True):
    key = "nc" if spmd else "nc_sim"
    if key in _prog_cache:
        return _prog_cache[key]

    nc = bacc.Bacc("TRN2", target_bir_lowering=False, debug=False,
                   num_devices=NC if spmd else 1)

    def _collective(kind, op, ins, outs):
        if spmd:
            nc.gpsimd.collective_compute(
                kind, op, replica_groups=[list(range(NC))], ins=ins, outs=outs,
            )
        else:
            i, o = ins[0], outs[0]
            n = min(i.size(), o.size())
            nc.gpsimd.dma_start(o[0:n], i[0:n])

    # ---- per-core external inputs ----
    d_cat1 = nc.dram_tensor("cat1_cols", [128, 16], F32, kind="ExternalInput")
    d_h0c8 = nc.dram_tensor("h0_cols8", [128, 8], F32, kind="ExternalInput")
    d_h0cc = nc.dram_tensor("h0_col_c", [128, 1], F32, kind="ExternalInput")
    d_aw = nc.dram_tensor("aw_tiled", [128, 16, MLS], F32, kind="ExternalInput")
    d_ab = nc.dram_tensor("attn_b", [1, MLS], F32, kind="ExternalInput")
    d_enc = nc.dram_tensor("enc_tiled", [128, 4, H], F32, kind="ExternalInput")
    d_cw = nc.dram_tensor("cw_tiled", [128, 16, HC], F32, kind="ExternalInput")
    d_cbc = nc.dram_tensor("comb_b_row", [1, HC], F32, kind="ExternalInput")
    d_wih = nc.dram_tensor("wih_cols", [128, 3 * H], F32, kind="ExternalInput")
    d_whh = nc.dram_tensor("whh_cols", [128, 3 * H], F32, kind="ExternalInput")
    d_bih = nc.dram_tensor("bih_cols", [128, 24], F32, kind="ExternalInput")
    d_bhh = nc.dram_tensor("bhh_cols", [128, 24], F32, kind="ExternalInput")
    d_owT = nc.dram_tensor("out_wT", [H, VS], F32, kind="ExternalInput")
    d_ob = nc.dram_tensor("out_b", [VS], F32, kind="ExternalInput")

    # ---- per-core external outputs ----
    o_attn = nc.dram_tensor("out_attn", [MLS], F32, kind="ExternalOutput")
    o_h = nc.dram_tensor("out_h", [H], F32, kind="ExternalOutput")
    o_logp = nc.dram_tensor("out_logp", [VS], F32, kind="ExternalOutput")

    with tile.TileContext(nc) as tc:
        with (
            tc.tile_pool(name="stage", bufs=1) as sp,
            tc.tile_pool(name="small", bufs=1) as sm,
            tc.tile_pool(name="dram", bufs=1, space="DRAM") as dp,
            tc.tile_pool(name="lstage", bufs=6) as lp,
            tc.tile_pool(name="wpool", bufs=WPOOL_BUFS) as wp,
        ):
            # -------- latency-critical small loads (scalar HWDGE ring) ------
            cat1_sb = sp.tile([128, 16], F32R)
            nc.scalar.dma_start(cat1_sb[:], d_cat1[:].bitcast(F32R))
            h0c8_sb = sp.tile([128, 8], F32)
            nc.scalar.dma_start(h0c8_sb[:], d_h0c8[:])
            h0cc_sb = sp.tile([128, 1], F32R)
            nc.scalar.dma_start(h0cc_sb[:], d_h0cc[:].bitcast(F32R))
            ab_sb = sp.tile([1, MLS], F32)
            nc.scalar.dma_start(ab_sb[:], d_ab[:])
            cbc_sb = sp.tile([1, HC], F32)
            nc.scalar.dma_start(cbc_sb[:], d_cbc[:])
            bih_sb = sp.tile([128, 24], F32)
            nc.scalar.dma_start(bih_sb[:], d_bih[:])
            bhh_sb = sp.tile([128, 24], F32)
            nc.scalar.dma_start(bhh_sb[:], d_bhh[:])

            # -------- staged weight loads (sync HWDGE ring, contiguous) -----
            aw_c = []
            for jc in range(4):
                t = wp.tile([128, 4, MLS], F32R, tag="w", name=f"awc{jc}")
                nc.sync.dma_start(t[:], d_aw[:, jc * 4 : (jc + 1) * 4, :].bitcast(F32R))
                aw_c.append(t)
            enc_sb = sp.tile([128, 4, H], F32R)
            nc.sync.dma_start(enc_sb[:], d_enc[:].bitcast(F32R))
            cw_sb = wp.tile([128, 16, HC], F32R, tag="w", name="cw_sb")
            nc.sync.dma_start(cw_sb[:], d_cw[:].bitcast(F32R))
            whh_sb = sp.tile([128, 3 * H], F32R)
            nc.sync.dma_start(whh_sb[:], d_whh[:].bitcast(F32R))
            wih_sb = sp.tile([128, 3 * H], F32R)
            nc.sync.dma_start(wih_sb[:], d_wih[:].bitcast(F32R))

            # DRAM bounce buffers
            ar1_in = dp.tile([1056], F32)
            ar1_out = dp.tile([1056], F32, addr_space="Shared")
            ar2_in = dp.tile([6144], F32)
            ar2_out = dp.tile([6144], F32, addr_space="Shared")
            ar4_in = dp.tile([8], F32)
            ar4_out = dp.tile([8], F32, addr_space="Shared")
            scratch = dp.tile([VS], F32)

            with tc.tile_pool(name="psum14", bufs=1, space="PSUM") as pp:
                # ============ stage 1: attention scores ============
                psum1 = pp.tile([1, MLS], F32)
                for j in range(16):
                    nc.tensor.matmul(
                        psum1[:], cat1_sb[:, j : j + 1], aw_c[j // 4][:, j % 4, :],
                        start=(j == 0), stop=(j == 15),
                    )
                logits1 = sm.tile([1, MLS], F32)
                nc.vector.tensor_add(logits1[:], psum1[:], ab_sb[:])
                # softmax without max-subtraction (logits are O(1) by construction)
                unnorm = sm.tile([1, MLS], F32)
                d_part = sm.tile([1, 1], F32)
                nc.scalar.activation(unnorm[:], logits1[:], AF.Exp, accum_out=d_part[:])
                # repartition [1,512] -> [128,4] on the PE (transpose), no DMA hop
                ones11 = sm.tile([1, 1], F32)
                nc.vector.memset(ones11[:], 1.0)
                psum_u = pp.tile([128, 4], F32, tag="pT", bufs=1, name="psum_u")
                for i in range(4):
                    nc.tensor.transpose(
                        psum_u[:, i : i + 1], unnorm[0:1, i * 128 : (i + 1) * 128], ones11[:]
                    )
                unn_cols = sm.tile([128, 4], F32R)
                nc.vector.tensor_copy(unn_cols[:], psum_u[:])

                # ============ stage 2: attn_applied partials ============
                psum2a = pp.tile([1, 512], F32, tag="p2", bufs=2)
                psum2b = pp.tile([1, 512], F32, tag="p2", bufs=2)
                for mc in range(4):
                    nc.tensor.matmul(
                        psum2a[:], unn_cols[:, mc : mc + 1], enc_sb[:, mc, 0:512],
                        start=(mc == 0), stop=(mc == 3),
                    )
                for mc in range(4):
                    nc.tensor.matmul(
                        psum2b[:], unn_cols[:, mc : mc + 1], enc_sb[:, mc, 512:1024],
                        start=(mc == 0), stop=(mc == 3),
                    )
                ar1_sb = sm.tile([1, 1056], F32)
                nc.vector.memset(ar1_sb[:], 0.0)
                nc.vector.tensor_copy(ar1_sb[:, 0:512], psum2a[:])
                nc.vector.tensor_copy(ar1_sb[:, 512:1024], psum2b[:])
                nc.vector.tensor_copy(ar1_sb[:, 1024:1025], d_part[:])
                nc.gpsimd.dma_start(ar1_in[:].unsqueeze(0), ar1_sb[:])
                _collective("AllReduce", mybir.AluOpType.add, [ar1_in[:]], [ar1_out[:]])
                u_cols = sm.tile([128, 8], F32R)
                nc.gpsimd.dma_start(
                    u_cols[:], ar1_out[0:1024].rearrange("(j p) -> p j", p=128).bitcast(F32R)
                )
                dtot = sm.tile([1, 1], F32)
                nc.gpsimd.dma_start(dtot[:], ar1_out[1024:1025].unsqueeze(0))
                inv_d = sm.tile([1, 1], F32)
                nc.vector.reciprocal(inv_d[:], dtot[:])
                # attn_weights output = unnorm / d_total
                aw_out = sm.tile([1, MLS], F32)
                nc.vector.tensor_scalar_mul(aw_out[:], unnorm[:], inv_d[:])
                nc.scalar.dma_start(o_attn[:].unsqueeze(0), aw_out[:])

                # ============ stage 3: combine + relu -> x chunk [1,128] ====
                psumA = pp.tile([1, HC], F32, tag="pAB", bufs=2)
                psumB = pp.tile([1, HC], F32, tag="pAB", bufs=2)
                for j in range(8):
                    nc.tensor.matmul(
                        psumA[:], cat1_sb[:, j : j + 1], cw_sb[:, j, :],
                        start=(j == 0), stop=(j == 7),
                    )
                for j in range(8):
                    nc.tensor.matmul(
                        psumB[:], u_cols[:, j : j + 1], cw_sb[:, j + 8, :],
                        start=(j == 0), stop=(j == 7),
                    )
                xt = sm.tile([1, HC], F32)
                nc.vector.tensor_scalar_mul(xt[:], psumB[:], inv_d[:])
                nc.vector.tensor_add(xt[:], xt[:], psumA[:])
                nc.vector.tensor_add(xt[:], xt[:], cbc_sb[:])
                xr = sm.tile([1, HC], F32)
                nc.vector.tensor_scalar_max(xr[:], xt[:], 0.0)
                # repartition x [1,128] -> [128,1] on the PE (transpose)
                psum_x = pp.tile([128, 1], F32, tag="pT", bufs=1, name="psum_x")
                nc.tensor.transpose(psum_x[:], xr[:], ones11[:])
                x4 = sm.tile([128, 1], F32R)
                nc.vector.tensor_copy(x4[:], psum_x[:])

                # ============ stage 4a: gi/gh partial sums ============
                ar2h_gh = sm.tile([1, 3072], F32, tag="ar2h", bufs=1)
                for s in range(6):
                    pg = pp.tile([1, 512], F32, name=f"psum_gh{s}", tag="pg", bufs=2)
                    nc.tensor.matmul(
                        pg[:], h0cc_sb[:], whh_sb[:, s * 512 : (s + 1) * 512],
                        start=True, stop=True,
                    )
                    eng = nc.vector.tensor_copy if s % 2 == 0 else nc.scalar.copy
                    eng(ar2h_gh[:, s * 512 : (s + 1) * 512], pg[:])
                nc.gpsimd.dma_start(ar2_in[3072:6144].unsqueeze(0), ar2h_gh[:])
                ar2h_gi = sm.tile([1, 3072], F32, tag="ar2h", bufs=1)
                for s in range(6):
                    pg = pp.tile([1, 512], F32, name=f"psum_gi{s}", tag="pg", bufs=2)
                    nc.tensor.matmul(
                        pg[:], x4[:], wih_sb[:, s * 512 : (s + 1) * 512],
                        start=True, stop=True,
                    )
                    eng = nc.vector.tensor_copy if s % 2 == 0 else nc.scalar.copy
                    eng(ar2h_gi[:, s * 512 : (s + 1) * 512], pg[:])
                nc.gpsimd.dma_start(ar2_in[0:3072].unsqueeze(0), ar2h_gi[:])
                _collective("AllReduce", mybir.AluOpType.add, [ar2_in[:]], [ar2_out[:]])

                # ============ stage 4b: gates, full h_new on every core =====
                giC = sm.tile([128, 24], F32)
                nc.gpsimd.dma_start(
                    giC[:], ar2_out[0:3072].rearrange("(j p) -> p j", p=128)
                )
                ghC = sm.tile([128, 24], F32)
                nc.gpsimd.dma_start(
                    ghC[:], ar2_out[3072:6144].rearrange("(j p) -> p j", p=128)
                )
                nc.vector.tensor_add(giC[:], giC[:], bih_sb[:])
                nc.vector.tensor_add(ghC[:], ghC[:], bhh_sb[:])
                ga = sm.tile([128, 8], F32)
                gt = sm.tile([128, 8], F32)
                r_g = sm.tile([128, 8], F32)
                z_g = sm.tile([128, 8], F32)
                n_g = sm.tile([128, 8], F32)
                # sigmoid(a) = 0.5*tanh(a/2)+0.5 (keeps the ACT table on Tanh)
                nc.vector.tensor_add(ga[:], giC[:, 0:8], ghC[:, 0:8])
                nc.scalar.activation(gt[:], ga[:], AF.Tanh, scale=0.5)
                nc.vector.tensor_scalar(r_g[:], gt[:], 0.5, 0.5, mybir.AluOpType.mult, mybir.AluOpType.add)
                nc.vector.tensor_add(ga[:], giC[:, 8:16], ghC[:, 8:16])
                nc.scalar.activation(gt[:], ga[:], AF.Tanh, scale=0.5)
                nc.vector.tensor_scalar(z_g[:], gt[:], 0.5, 0.5, mybir.AluOpType.mult, mybir.AluOpType.add)
                nc.vector.tensor_mul(ga[:], r_g[:], ghC[:, 16:24])
                nc.vector.tensor_add(ga[:], ga[:], giC[:, 16:24])
                nc.scalar.activation(n_g[:], ga[:], AF.Tanh)
                # h_new = n + z*(h0 - n)
                hn_t = sm.tile([128, 8], F32)
                nc.vector.tensor_sub(hn_t[:], h0c8_sb[:], n_g[:])
                nc.vector.tensor_mul(hn_t[:], z_g[:], hn_t[:])
                h_cols = sm.tile([128, 8], F32R)
                nc.vector.tensor_add(h_cols[:], n_g[:], hn_t[:])
                nc.scalar.dma_start(
                    o_h[:].rearrange("(j p) -> p j", p=128), h_cols[:].bitcast(F32)
                )

            # ============ stage 5: out-projection (the big stream) ============
            L128 = sm.tile([128, VS // 128], F32)
            B128 = sm.tile([128, VS // 128], F32)
            E128 = sm.tile([128, VS // 128], F32)
            esum = sm.tile([128, 1], F32)
            nc.scalar.dma_start(B128[:], d_ob[:].rearrange("(p j) -> p j", p=128))
            wdmas = []
            with tc.tile_pool(name="psum5", bufs=PSUM_O_BUFS, space="PSUM") as pp5:
                for vb in range(VBLK):
                    wts = []
                    for k in range(8):
                        w_t = wp.tile([128, VTB * VT], F32R, tag="w")
                        eng = nc.sync if k % 2 == 0 else nc.scalar
                        wd = eng.dma_start(
                            w_t[:],
                            d_owT[k * 128 : (k + 1) * 128,
                                  vb * VTB * VT : (vb + 1) * VTB * VT].bitcast(F32R),
                        )
                        # Cap the DMA-queue backlog while the serial attention/GRU
                        # chain still needs low-latency small transfers: the first
                        # GATED stream DMAs are chained pairwise (<=2 in flight).
                        GATED = 20
                        if len(wdmas) >= 2 and len(wdmas) < GATED:
                            add_dep_helper(
                                wd.ins, wdmas[-2].ins, sync=True,
                                reason="cap stream backlog during serial chain",
                            )
                        wdmas.append(wd)
                        wts.append(w_t)
                    for i in range(VTB):
                        v = vb * VTB + i
                        psum_o = pp5.tile([1, VT], F32, tag="po")
                        for k in range(8):
                            nc.tensor.matmul(
                                psum_o[:],
                                h_cols[:, k : k + 1],
                                wts[k][:, i * VT : (i + 1) * VT],
                                start=(k == 0), stop=(k == 7),
                            )
                        logit_sb = lp.tile([1, VT], F32, tag="ls")
                        nc.vector.tensor_copy(logit_sb[:], psum_o[:])
                        nc.scalar.dma_start(
                            scratch[v * VT : (v + 1) * VT].unsqueeze(0), logit_sb[:]
                        )
                    # incremental tail: after every second superblock, pull the
                    # pair's logits back as 32 partition rows (32-aligned), add
                    # bias, exp + accumulate the sum
                    if vb % 2 == 1:
                        rs = slice((vb - 1) * 16, (vb + 1) * 16)
                        nc.scalar.dma_start(
                            L128[rs, :],
                            scratch[(vb - 1) * VTB * VT : (vb + 1) * VTB * VT]
                            .rearrange("(p j) -> p j", p=32),
                        )
                        nc.vector.tensor_add(L128[rs, :], L128[rs, :], B128[rs, :])
                        nc.scalar.activation(
                            E128[rs, :], L128[rs, :], AF.Exp, accum_out=esum[rs, :]
                        )

            # ============ tail: log-softmax over full vocab ============
            with tc.tile_pool(name="psumT", bufs=1, space="PSUM") as ppt:
                ones_col = sm.tile([128, 1], F32)
                nc.vector.memset(ones_col[:], 1.0)
                psum_s = ppt.tile([1, 1], F32)
                nc.tensor.matmul(psum_s[:], ones_col[:], esum[:], start=True, stop=True)
                ar4_sb = sm.tile([1, 8], F32)
                nc.vector.memset(ar4_sb[:], 0.0)
                nc.vector.tensor_copy(ar4_sb[:, 0:1], psum_s[:])
                nc.gpsimd.dma_start(ar4_in[:].unsqueeze(0), ar4_sb[:])
                _collective("AllReduce", mybir.AluOpType.add, [ar4_in[:]], [ar4_out[:]])
                stot = sm.tile([1, 1], F32)
                nc.gpsimd.dma_start(stot[:], ar4_out[0:1].unsqueeze(0))
                lse = sm.tile([1, 1], F32)
                nc.scalar.activation(lse[:], stot[:], AF.Ln)
                ones_row2 = sm.tile([1, 128], F32)
                nc.vector.memset(ones_row2[:], 1.0)
                psum_b = ppt.tile([128, 1], F32)
                nc.tensor.matmul(psum_b[:], ones_row2[:], lse[:], start=True, stop=True)
                lse128 = sm.tile([128, 1], F32)
                nc.vector.tensor_copy(lse128[:], psum_b[:])
                OUT128 = sm.tile([128, VS // 128], F32)
                nc.vector.tensor_scalar(
                    OUT128[:], L128[:], lse128[:], None, mybir.AluOpType.subtract
                )
                nc.scalar.dma_start(
                    o_logp[:].rearrange("(p j) -> p j", p=128), OUT128[:]
                )

    nc.compile()
    _prog_cache[key] = nc
    return nc


def make_in_maps(
    input_ids, hidden, encoder_outputs, emb, attn_w, attn_b,
    comb_w, comb_b, w_ih, w_hh, b_ih, b_hh, out_w, out_b,
):
    f = np.float32
    idx = int(np.asarray(input_ids).reshape(-1)[0])
    e0 = np.asarray(emb[idx], f).reshape(H)
    h0 = np.asarray(hidden, f).reshape(H)
    cat1 = np.concatenate([e0, h0])
    cat1_cols = np.ascontiguousarray(cat1.reshape(16, 128).T)  # [128,16]
    h0_cols8 = np.ascontiguousarray(h0.reshape(8, 128).T)      # [128,8]

    attn_w = np.asarray(attn_w, f)
    attn_b = np.asarray(attn_b, f)
    enc = np.asarray(encoder_outputs, f)
    comb_w = np.asarray(comb_w, f)
    comb_b = np.asarray(comb_b, f)
    w_ih = np.asarray(w_ih, f)
    w_hh = np.asarray(w_hh, f)
    b_ih = np.asarray(b_ih, f)
    b_hh = np.asarray(b_hh, f)
    out_w = np.asarray(out_w, f)
    out_b = np.asarray(out_b, f)

    # replicated gate-bias tiles [128, 24]: (p, j) = b[j*128+p]
    bih_cols = np.ascontiguousarray(b_ih.reshape(24, 128).T)
    bhh_cols = np.ascontiguousarray(b_hh.reshape(24, 128).T)

    in_maps = []
    for c in range(NC):
        ms = slice(c * MLS, (c + 1) * MLS)
        hs = slice(c * HC, (c + 1) * HC)
        # [128,16,512]: (p, j, m) = attn_w[c*512+m, j*128+p]
        aw_tiled = np.ascontiguousarray(
            attn_w[ms].reshape(MLS, 16, 128).transpose(2, 1, 0)
        )
        # [128,4,1024]: (p, mc, h) = enc[c*512 + mc*128 + p, h]
        enc_tiled = np.ascontiguousarray(
            enc[ms].reshape(4, 128, H).transpose(1, 0, 2)
        )
        # [128,16,128]: (p, j, n) = comb_w[c*128+n, j*128+p]
        cw_tiled = np.ascontiguousarray(
            comb_w[hs].reshape(HC, 16, 128).transpose(2, 1, 0)
        )
        # [128, 3072]: (p, t) = w[t, c*128+p]
        wih_cols = np.ascontiguousarray(w_ih[:, hs].T)
        whh_cols = np.ascontiguousarray(w_hh[:, hs].T)
        in_maps.append({
            "cat1_cols": cat1_cols,
            "h0_cols8": h0_cols8,
            "h0_col_c": np.ascontiguousarray(h0[hs].reshape(HC, 1)),
            "aw_tiled": aw_tiled,
            "attn_b": attn_b[ms].reshape(1, MLS),
            "enc_tiled": enc_tiled,
            "cw_tiled": cw_tiled,
            "comb_b_row": np.ascontiguousarray(comb_b[hs].reshape(1, HC)),
            "wih_cols": wih_cols,
            "whh_cols": whh_cols,
            "bih_cols": bih_cols,
            "bhh_cols": bhh_cols,
            "out_wT": np.ascontiguousarray(out_w[c * VS : (c + 1) * VS].T),
            "out_b": np.ascontiguousarray(out_b[c * VS : (c + 1) * VS]),
        })
    return in_maps


def kernel(**inputs):
    nc = build_program()
    in_maps = make_in_maps(**inputs)
    res = run_bass_kernel_spmd(nc, in_maps, list(range(NC)))
    out = np.concatenate([res.results[c]["out_logp"] for c in range(NC)]).reshape(1, V)
    h_new = np.asarray(res.results[0]["out_h"]).reshape(1, 1, H)
    attn = np.concatenate([res.results[c]["out_attn"] for c in range(NC)]).reshape(1, ML)
    return out, h_new, attn
